# revision 1
# baseline (speedup 1.0000x reference)
"""DiT block kernel for 8 TRN2 NeuronCores (self-contained).

Sharding: cores 0-3 <-> batch 0, cores 4-7 <-> batch 1.
Per 4-core group: attention head-parallel (3 of 12 heads/core, all 2048
tokens), W_o row-sharded -> ReduceScatter -> each core owns a 512-token
slice; FFN token-parallel (512 rows, full weights). AdaLN/cond path is
DH-sharded over all 8 cores with host-folded (cond_w2 @ W_mod) matrices
-> one small AllReduce. Matmuls run float32r (full PE speed at N>=256).
"""
import numpy as np

import concourse.bass as bass
import concourse.mybir as mybir
import concourse.tile as tile
from concourse import bacc, bass_utils
from concourse.masks import make_identity

FP32 = mybir.dt.float32
FP32R = mybir.dt.float32r
AF = mybir.ActivationFunctionType
ALU = mybir.AluOpType
AX = mybir.AxisListType

B, L, D, H, DH = 2, 2048, 768, 12, 3072
HD = 64
EPS = 1e-6
SCALE = float(np.sqrt(HD))
NC_ = 8
G = 4            # cores per batch group
HC = 3           # heads per core
TOK = L // G     # 512
DH8 = DH // NC_  # 384
GROUPS = [[0, 1, 2, 3], [4, 5, 6, 7]]
KC = L // 128    # 16 key chunks
JT = L // 512    # 4 q tiles
DK = D // 128    # 6 d chunks
MG = DH // 128   # 24 dh chunks


# ---------------------------------------------------------------- host prep
def host_prep(inp):
    f = {k: np.ascontiguousarray(np.asarray(v, np.float32)) for k, v in inp.items()}
    x, c = f["x"], f["c"]
    cos, sin = f["freqs_cos"], f["freqs_sin"]          # [L, 32]

    attn_gamma_s = f["attn_gamma"] * f["attn_norm_w"][None, :]
    ffn_gamma_s = f["ffn_gamma"] * f["ffn_norm_w"][None, :]
    mods = [attn_gamma_s, f["attn_beta"], f["attn_alpha"],
            ffn_gamma_s, f["ffn_beta"], f["ffn_gamma"]]
    wfold_full = [f["cond_w2"] @ m for m in mods]       # [DH, D] x6
    bvec = [f["cond_b2"] @ m for m in mods]             # [D] x6

    perm = np.concatenate([np.arange(0, HD, 2), np.arange(1, HD, 2)])
    cosT, sinT = cos.T, sin.T                            # [32, L]
    cct = np.tile(cosT, (4, 1)).astype(np.float32)       # [128, L]
    sst = np.concatenate([-sinT, sinT, -sinT, sinT], 0).astype(np.float32)

    # bvec pack [128, 72] batch-major: col b*36 + m*6+oc = bvec[m][oc*128+p]
    bvp = np.zeros((128, 72), np.float32)
    for m in range(6):
        for oc in range(6):
            col = m * 6 + oc
            bvp[:, col] = bvec[m][oc * 128:(oc + 1) * 128]
            bvp[:, 36 + col] = bvp[:, col]

    cT = np.ascontiguousarray(c.T)                       # [768, 2]
    ct_pack = cT.reshape(6, 128, 2).transpose(1, 0, 2).reshape(128, 12).copy()

    wg_blk = f["ffn_gate"].reshape(6, 128, 24, 128).transpose(2, 1, 0, 3) \
        .reshape(24 * 128, 768).copy()
    wh_blk = f["ffn_hidden"].reshape(6, 128, 24, 128).transpose(2, 1, 0, 3) \
        .reshape(24 * 128, 768).copy()

    pswap = np.zeros((128, 128), np.float32)
    for i_ in range(128):
        pswap[i_, i_ ^ 32] = 1.0

    cores = []
    for i in range(NC_):
        g, r = i // G, i % G
        hs = [HC * r + j for j in range(HC)]
        si = slice(DH8 * i, DH8 * (i + 1))
        blocks = [f["W_q"][:, h * HD:(h + 1) * HD][:, perm] for h in hs]
        blocks += [f["W_k"][:, h * HD:(h + 1) * HD][:, perm] for h in hs]
        wqk = np.ascontiguousarray(np.concatenate(blocks, 1))    # [768, 384]
        wv = np.zeros((D, 256), np.float32)
        for j, h in enumerate(hs):
            wv[:, j * HD:(j + 1) * HD] = f["W_v"][:, h * HD:(h + 1) * HD]
        wo = np.ascontiguousarray(
            np.concatenate([f["W_o"][h * HD:(h + 1) * HD] for h in hs], 0))
        cores.append(dict(
            xT=np.ascontiguousarray(x[g].T),
            x_slice=np.ascontiguousarray(x[g, TOK * r:TOK * (r + 1)]),
            cct=cct, sst=sst, ct_pack=ct_pack,
            w1s=np.ascontiguousarray(f["cond_w1"][:, si]),
            b1_pack=np.ascontiguousarray(f["cond_b1"][si].reshape(3, 128).T),
            wfold=np.ascontiguousarray(
                np.concatenate([w[si] for w in wfold_full], 0)),
            bvec_pack=bvp,
            gflag=np.full((128, 1), float(g), np.float32),
            pswap=pswap,
            wqk=wqk, wv=wv, wo=wo,
            wg_blk=wg_blk, wh_blk=wh_blk, wout=f["ffn_out"],
        ))
    return cores


# ---------------------------------------------------------------- program
_CACHE = {}

DRAM_SPECS = [
    ("xT", [D, L], FP32),
    ("x_slice", [TOK, D], FP32),
    ("cct", [128, L], FP32),
    ("sst", [128, L], FP32),
    ("ct_pack", [128, 12], FP32R),
    ("w1s", [D, DH8], FP32R),
    ("b1_pack", [128, 3], FP32),
    ("wfold", [6 * DH8, D], FP32R),
    ("bvec_pack", [128, 72], FP32),
    ("gflag", [128, 1], FP32),
    ("pswap", [128, 128], FP32R),
    ("wqk", [D, 384], FP32R),
    ("wv", [D, 256], FP32R),
    ("wo", [HC * HD, D], FP32R),
    ("wg_blk", [MG * 128, D], FP32R),
    ("wh_blk", [MG * 128, D], FP32R),
    ("wout", [DH, D], FP32R),
]


def build_program(reps=1):
    nc = bacc.Bacc("TRN2", target_bir_lowering=False, debug=False,
                   num_devices=NC_)
    dr = {}
    for name, shape, dt in DRAM_SPECS:
        dr[name] = nc.dram_tensor(name, shape, dt, kind="ExternalInput")
    out_d = nc.dram_tensor("out", [TOK, D], FP32, kind="ExternalOutput")

    with tile.TileContext(nc) as tc, \
         nc.allow_low_precision(reason="float32r matmul operands (same bits as fp32)"):
        for _ in range(reps):
            _emit(nc, tc, dr, out_d)
    nc.compile()
    return nc


def _phase_a(nc, tc, dr, pers, st):
    """cond MLP + mod vectors -> st['mod_g'], st['bcast']."""
    mod_g, bcast = st["mod_g"], st["bcast"]
    with tc.tile_pool(name="pa", bufs=1) as pa, \
         tc.tile_pool(name="pa_wf", bufs=3) as pa_wf:
        ct_sb = pa.tile([128, 12], FP32R, name="ct_sb")
        nc.sync.dma_start(ct_sb[:], dr["ct_pack"].ap())
        b1_sb = pa.tile([128, 3], FP32, name="b1_sb")
        nc.sync.dma_start(b1_sb[:], dr["b1_pack"].ap())
        w1_sb = [pa.tile([128, DH8], FP32R, name=f"w1_sb{k}")
                 for k in range(DK)]
        for k in range(DK):
            nc.sync.dma_start(w1_sb[k][:],
                              dr["w1s"].ap()[128 * k:128 * (k + 1), :])
        bv_sb = pa.tile([128, 72], FP32, name="bv_sb")
        nc.sync.dma_start(bv_sb[:], dr["bvec_pack"].ap())

        silu_sb = [pa.tile([128, 2], FP32R, name=f"silu{m}") for m in range(3)]
        with tc.tile_pool(name="pa_ps1", bufs=2, space="PSUM") as pa_ps1:
            for m in range(3):
                h1_ps = pa_ps1.tile([128, 2], FP32, name="h1_ps")
                for k in range(DK):
                    nc.tensor.matmul(h1_ps[:],
                                     w1_sb[k][:, 128 * m:128 * (m + 1)],
                                     ct_sb[:, 2 * k:2 * k + 2],
                                     start=(k == 0), stop=(k == DK - 1))
                nc.scalar.activation(silu_sb[m][:], h1_ps[:], AF.Silu,
                                     bias=b1_sb[:, m:m + 1])

        arin_sb = pa.tile([128, 72], FP32, name="arin_sb")
        with tc.tile_pool(name="pa_ps2", bufs=1, space="PSUM") as pa_ps2:
            for m in range(6):
                mp = [pa_ps2.tile([128, 2], FP32, name=f"modp{oc}")
                      for oc in range(6)]
                for k in range(3):
                    wf = pa_wf.tile([128, D], FP32R, name="wf")
                    nc.sync.dma_start(
                        wf[:], dr["wfold"].ap()[128 * (3 * m + k):
                                                128 * (3 * m + k + 1), :])
                    for oc in range(6):
                        nc.tensor.matmul(mp[oc][:],
                                         wf[:, 128 * oc:128 * (oc + 1)],
                                         silu_sb[k][:],
                                         start=(k == 0), stop=(k == 2))
                for oc in range(6):
                    col = m * 6 + oc
                    nc.vector.tensor_copy(arin_sb[:, col:col + 1],
                                          mp[oc][:, 0:1])
                    nc.vector.tensor_copy(arin_sb[:, 36 + col:37 + col],
                                          mp[oc][:, 1:2])
        nc.sync.dma_start(st["ar_in"][:], arin_sb[:])
        nc.gpsimd.collective_compute(
            "AllReduce", ALU.add, replica_groups=[list(range(NC_))],
            ins=[st["ar_in"].opt()], outs=[st["ar_out"].opt()])
        mod_sb = pa.tile([128, 72], FP32, name="mod_sb")
        nc.sync.dma_start(mod_sb[:], st["ar_out"][:])
        nc.vector.tensor_add(mod_sb[:], mod_sb[:], bv_sb[:])
        d1 = pa.tile([128, 36], FP32, name="d1")
        nc.vector.tensor_sub(d1[:], mod_sb[:, 36:72], mod_sb[:, 0:36])
        nc.vector.scalar_tensor_tensor(
            mod_g[:], d1[:], st["gflag_sb"][:, 0:1], mod_sb[:, 0:36],
            op0=ALU.mult, op1=ALU.add)

        with tc.tile_pool(name="pa_bc", bufs=2, space="PSUM") as pa_bc:
            for m in (2, 3, 4, 5):
                mrow = pa.tile([1, D], FP32R, name=f"mrow{m}")
                for half in range(2):
                    trp = pa_bc.tile([1, 384], FP32, name="trp")
                    for j in range(3):
                        oc = half * 3 + j
                        nc.tensor.transpose(
                            trp[0:1, 128 * j:128 * (j + 1)],
                            mod_g[:, m * 6 + oc:m * 6 + oc + 1], st["ident"][:])
                    nc.vector.tensor_copy(
                        mrow[0:1, 384 * half:384 * (half + 1)], trp[:])
                for half in range(2):
                    bps = pa_bc.tile([128, 384], FP32, name="bps")
                    nc.tensor.matmul(
                        bps[:], st["ones_r"][:],
                        mrow[0:1, 384 * half:384 * (half + 1)])
                    nc.vector.tensor_copy(
                        bcast[m][:, 384 * half:384 * (half + 1)], bps[:])


def _phase_b(nc, tc, dr, st, hT):
    """xT load + rms stats + hT (modulated normed x, transposed)."""
    mod_g = st["mod_g"]
    with tc.tile_pool(name="pb_x", bufs=1) as pb_x, \
         tc.tile_pool(name="pb", bufs=2) as pb, \
         tc.tile_pool(name="pb_ps", bufs=1, space="PSUM") as pb_ps:
        xt = [pb_x.tile([128, L], FP32, name=f"xt{k}") for k in range(DK)]
        for k in range(DK):
            nc.sync.dma_start(xt[k][:],
                              dr["xT"].ap()[128 * k:128 * (k + 1), :])
        msq = [pb_ps.tile([1, 512], FP32, name=f"msq{j}") for j in range(4)]
        for k in range(DK):
            for j in range(4):
                xsq = pb.tile([128, 512], FP32R, name="xsq")
                nc.vector.tensor_mul(xsq[:], xt[k][:, 512 * j:512 * (j + 1)],
                                     xt[k][:, 512 * j:512 * (j + 1)])
                nc.tensor.matmul(msq[j][:], st["onescol_r"][:], xsq[:],
                                 start=(k == 0), stop=(k == DK - 1))
        rb = pb.tile([128, L], FP32, name="rb", bufs=1)
        for j in range(4):
            sq_sb = pb.tile([1, 512], FP32, name="sq_sb")
            nc.scalar.activation(sq_sb[:], msq[j][:], AF.Sqrt,
                                 bias=st["eps_sb"][0:1, 0:1], scale=1.0 / D)
            rinv = pb.tile([1, 512], FP32R, name="rinv")
            nc.vector.reciprocal(rinv[:], sq_sb[:])
            rbp = pb_ps.tile([128, 512], FP32, name="rbp", bufs=2)
            nc.tensor.matmul(rbp[:], st["ones_r"][:], rinv[:])
            nc.vector.tensor_copy(rb[:, 512 * j:512 * (j + 1)], rbp[:])
        for k in range(DK):
            for j in range(4):
                sl = slice(512 * j, 512 * (j + 1))
                tmp = pb.tile([128, 512], FP32, name="tmp")
                nc.vector.tensor_mul(tmp[:], xt[k][:, sl], rb[:, sl])
                nc.vector.tensor_scalar(
                    hT[k][:, sl], tmp[:], mod_g[:, k:k + 1],
                    mod_g[:, 6 + k:7 + k], op0=ALU.mult, op1=ALU.add)


def _phase_c(nc, tc, dr, st, hT, qkr, v_sb):
    """QKV matmuls + RoPE + v_aug tiles."""
    with tc.tile_pool(name="pc_w", bufs=1) as pc_w, \
         tc.tile_pool(name="pc", bufs=2) as pc, \
         tc.tile_pool(name="pc_ps", bufs=2, space="PSUM") as pc_ps:
        wqk_sb = [pc_w.tile([128, 384], FP32R, name=f"wqk{k}")
                  for k in range(DK)]
        wv_sb = [pc_w.tile([128, 256], FP32R, name=f"wv{k}")
                 for k in range(DK)]
        for k in range(DK):
            nc.sync.dma_start(wqk_sb[k][:],
                              dr["wqk"].ap()[128 * k:128 * (k + 1), :])
            nc.sync.dma_start(wv_sb[k][:],
                              dr["wv"].ap()[128 * k:128 * (k + 1), :])
        cct_sb = pc_w.tile([128, L], FP32, name="cct_sb")
        sst_sb = pc_w.tile([128, L], FP32, name="sst_sb")
        nc.sync.dma_start(cct_sb[:], dr["cct"].ap())
        nc.sync.dma_start(sst_sb[:], dr["sst"].ap())
        pswap_sb = pc_w.tile([128, 128], FP32R, name="pswap_sb")
        nc.sync.dma_start(pswap_sb[:], dr["pswap"].ap())

        for m in range(3):
            qk_sb_m = pc.tile([128, L], FP32R, name="qk_sb", bufs=2)
            for j in range(4):
                qkp = pc_ps.tile([128, 512], FP32, name="qkp")
                for k in range(DK):
                    nc.tensor.matmul(qkp[:],
                                     wqk_sb[k][:, 128 * m:128 * (m + 1)],
                                     hT[k][:, 512 * j:512 * (j + 1)],
                                     start=(k == 0), stop=(k == DK - 1))
                nc.vector.tensor_copy(qk_sb_m[:, 512 * j:512 * (j + 1)],
                                      qkp[:])
            t1 = pc.tile([128, L], FP32, name="t1", bufs=1)
            t2 = pc.tile([128, L], FP32, name="t2", bufs=1)
            nc.vector.tensor_mul(t1[:], qk_sb_m[:].bitcast(FP32), cct_sb[:])
            for j in range(4):
                sl = slice(512 * j, 512 * (j + 1))
                swp = pc_ps.tile([128, 512], FP32, name="swp")
                nc.tensor.matmul(swp[:], pswap_sb[:], qk_sb_m[:, sl])
                nc.vector.tensor_mul(t2[:, sl], swp[:], sst_sb[:, sl])
            qkr_A, qkr_B, qkr_C, qkr_D = qkr
            if m == 0:      # (q0, q1) -> A full
                nc.vector.tensor_add(qkr_A[:], t1[:], t2[:])
            elif m == 1:    # (q2, k0) -> C[0:64], B[0:64]
                nc.vector.tensor_add(qkr_C[:], t1[0:64, :], t2[0:64, :])
                nc.vector.tensor_add(qkr_B[0:64, :], t1[64:128, :],
                                     t2[64:128, :])
            else:           # (k1, k2) -> B[64:128], D[0:64]
                nc.vector.tensor_add(qkr_B[64:128, :], t1[0:64, :],
                                     t2[0:64, :])
                nc.vector.tensor_add(qkr_D[:], t1[64:128, :],
                                     t2[64:128, :])

        for t in range(KC):
            vp = pc_ps.tile([128, 256], FP32, name="vp")
            for k in range(DK):
                nc.tensor.matmul(vp[:], hT[k][:, 128 * t:128 * (t + 1)],
                                 wv_sb[k][:],
                                 start=(k == 0), stop=(k == DK - 1))
            for h in range(HC):
                nc.vector.tensor_copy(v_sb[t][:, 65 * h:65 * h + 64],
                                      vp[:, 64 * h:64 * (h + 1)])
            nc.scalar.activation(v_sb[t][:, 64:195:65], st["ones3_f"][:],
                                 AF.Copy)


def _attention(nc, tc, st, qkr, v_sb, wo_sb, rs_in):
    """scoresT -> exp -> PV (ones-augmented) -> normalize -> Wo -> rs_in."""
    bcast = st["bcast"]
    with tc.tile_pool(name="at", bufs=2) as at, \
         tc.tile_pool(name="at_exp", bufs=6) as at_exp, \
         tc.tile_pool(name="at_sps", bufs=3, space="PSUM") as at_sps, \
         tc.tile_pool(name="at_ops", bufs=2, space="PSUM") as at_ops, \
         tc.tile_pool(name="at_bps", bufs=1, space="PSUM") as at_bps, \
         tc.tile_pool(name="at_wps", bufs=2, space="PSUM") as at_wps:
        qkr_A, qkr_B, qkr_C, qkr_D = qkr
        qsl = [lambda s: qkr_A[0:64, s], lambda s: qkr_A[64:128, s],
               lambda s: qkr_C[0:64, s]]
        ksl = [lambda s: qkr_B[0:64, s], lambda s: qkr_B[64:128, s],
               lambda s: qkr_D[0:64, s]]
        for j in range(JT):
            o_sb = [at.tile([64, 512], FP32R, name=f"o{h}")
                    for h in range(HC)]
            for h in range(HC):
                opsum = at_ops.tile([128, 512], FP32, name="opsum")
                for kc in range(KC):
                    sps = at_sps.tile([128, 512], FP32, name="sps")
                    nc.tensor.matmul(
                        sps[:],
                        ksl[h](slice(128 * kc, 128 * (kc + 1))),
                        qsl[h](slice(512 * j, 512 * (j + 1))))
                    ex = at_exp.tile([128, 512], FP32R, name="ex")
                    nc.scalar.activation(ex[:], sps[:], AF.Exp,
                                         scale=1.0 / SCALE)
                    nc.tensor.matmul(
                        opsum[0:65, :], v_sb[kc][:, 65 * h:65 * (h + 1)],
                        ex[:], start=(kc == 0), stop=(kc == KC - 1))
                recip = at.tile([1, 512], FP32R, name="recip")
                nc.vector.reciprocal(recip[:], opsum[64:65, :])
                bps = at_bps.tile([128, 512], FP32, name="bps")
                nc.tensor.matmul(bps[0:64, :], st["ones_r"][0:1, 0:64],
                                 recip[:])
                rb64 = at.tile([64, 512], FP32, name="rb64")
                nc.vector.tensor_copy(rb64[:], bps[0:64, :])
                nc.vector.tensor_mul(o_sb[h][:], opsum[0:64, :], rb64[:])
            for tc4 in range(4):
                tok = 512 * j + 128 * tc4
                out1 = at.tile([128, D], FP32, name="out1", bufs=3)
                for half in range(2):
                    wps = at_wps.tile([128, 384], FP32, name="wps")
                    for h in range(HC):
                        nc.tensor.matmul(
                            wps[:],
                            o_sb[h][:, 128 * tc4:128 * (tc4 + 1)],
                            wo_sb[h][:, 384 * half:384 * (half + 1)],
                            start=(h == 0), stop=(h == HC - 1))
                    nc.vector.tensor_mul(
                        out1[:, 384 * half:384 * (half + 1)], wps[:],
                        bcast[2][:, 384 * half:384 * (half + 1)])
                nc.sync.dma_start(rs_in[tok:tok + 128, :], out1[:])


def _ffn(nc, tc, dr, st, x1, out_d):
    """Token-parallel SwiGLU FFN + gated residual."""
    bcast, ident = st["bcast"], st["ident"]
    with tc.tile_pool(name="pf_h", bufs=1) as pf_h, \
         tc.tile_pool(name="pf", bufs=2) as pf, \
         tc.tile_pool(name="pf_gh", bufs=1) as pf_gh, \
         tc.tile_pool(name="pf_w", bufs=3) as pf_w:
        h2t = [pf_h.tile([128, TOK], FP32R, name=f"h2t{k}") for k in range(DK)]
        with tc.tile_pool(name="pf_tps", bufs=2, space="PSUM") as pf_tps:
            for t in range(4):
                sq = pf.tile([128, D], FP32, name="sq")
                nc.vector.tensor_mul(sq[:], x1[t][:], x1[t][:])
                ms = pf.tile([128, 1], FP32, name="ms")
                nc.vector.reduce_sum(ms[:], sq[:], axis=AX.X)
                sr = pf.tile([128, 1], FP32, name="sr")
                nc.scalar.activation(sr[:], ms[:], AF.Sqrt,
                                     bias=st["eps_sb"][:, 0:1], scale=1.0 / D)
                rv = pf.tile([128, 1], FP32, name="rv")
                nc.vector.reciprocal(rv[:], sr[:])
                h2a = pf.tile([128, D], FP32, name="h2a")
                nc.vector.tensor_scalar(h2a[:], x1[t][:], rv[:, 0:1], None,
                                        op0=ALU.mult)
                h2b = pf.tile([128, D], FP32, name="h2b")
                nc.vector.tensor_mul(h2b[:], h2a[:], bcast[3][:])
                h2c = pf.tile([128, D], FP32, name="h2c")
                nc.vector.tensor_add(h2c[:], h2b[:], bcast[4][:])
                for k in range(DK):
                    tp = pf_tps.tile([128, 128], FP32, name="tp")
                    nc.tensor.transpose(tp[:], h2c[:, 128 * k:128 * (k + 1)],
                                        ident[:])
                    nc.vector.tensor_copy(h2t[k][:, 128 * t:128 * (t + 1)],
                                          tp[:])

        ghT = [pf_gh.tile([128, TOK], FP32R, name=f"ghT{m}") for m in range(MG)]
        with tc.tile_pool(name="pf_gps", bufs=2, space="PSUM") as pf_gps:
            for m in range(MG):
                wg = pf_w.tile([128, D], FP32R, name="wg")
                wh = pf_w.tile([128, D], FP32R, name="wh")
                nc.sync.dma_start(
                    wg[:], dr["wg_blk"].ap()[128 * m:128 * (m + 1), :])
                nc.sync.dma_start(
                    wh[:], dr["wh_blk"].ap()[128 * m:128 * (m + 1), :])
                gp = pf_gps.tile([128, TOK], FP32, name="gp")
                hp = pf_gps.tile([128, TOK], FP32, name="hp")
                for k in range(DK):
                    nc.tensor.matmul(gp[:], wg[:, 128 * k:128 * (k + 1)],
                                     h2t[k][:],
                                     start=(k == 0), stop=(k == DK - 1))
                for k in range(DK):
                    nc.tensor.matmul(hp[:], wh[:, 128 * k:128 * (k + 1)],
                                     h2t[k][:],
                                     start=(k == 0), stop=(k == DK - 1))
                sg = pf.tile([128, TOK], FP32, name="sg")
                nc.scalar.activation(sg[:], gp[:], AF.Silu)
                nc.vector.tensor_mul(ghT[m][:], sg[:], hp[:])

        with tc.tile_pool(name="pf_ops", bufs=1, space="PSUM") as pf_ops:
            fps = [[pf_ops.tile([128, 384], FP32, name=f"fps{t}_{hf}")
                    for hf in range(2)] for t in range(4)]
            for k in range(MG):
                wo_t = pf_w.tile([128, D], FP32R, name="wout")
                nc.sync.dma_start(
                    wo_t[:], dr["wout"].ap()[128 * k:128 * (k + 1), :])
                for t in range(4):
                    for hf in range(2):
                        nc.tensor.matmul(
                            fps[t][hf][:],
                            ghT[k][:, 128 * t:128 * (t + 1)],
                            wo_t[:, 384 * hf:384 * (hf + 1)],
                            start=(k == 0), stop=(k == MG - 1))
            for t in range(4):
                ot = pf.tile([128, D], FP32, name="ot")
                for hf in range(2):
                    tt = pf.tile([128, 384], FP32, name="tt")
                    nc.vector.tensor_mul(tt[:], fps[t][hf][:],
                                         bcast[5][:, 384 * hf:384 * (hf + 1)])
                    nc.vector.tensor_add(ot[:, 384 * hf:384 * (hf + 1)],
                                         tt[:],
                                         x1[t][:, 384 * hf:384 * (hf + 1)])
                nc.sync.dma_start(out_d.ap()[128 * t:128 * (t + 1), :], ot[:])


def _emit(nc, tc, dr, out_d):
    with tc.tile_pool(name="pers", bufs=1) as pers, \
         tc.tile_pool(name="dram", bufs=1, space="DRAM") as dram:
        st = {}
        st["ident"] = pers.tile([128, 128], FP32, name="ident")
        make_identity(nc, st["ident"][:])
        ones_f = pers.tile([1, 128], FP32, name="ones_f")
        nc.vector.memset(ones_f[:], 1.0)
        st["ones_r"] = pers.tile([1, 128], FP32R, name="ones_r")
        nc.scalar.activation(st["ones_r"][:], ones_f[:], AF.Copy)
        onescol_f = pers.tile([128, 1], FP32, name="onescol_f")
        nc.vector.memset(onescol_f[:], 1.0)
        st["onescol_r"] = pers.tile([128, 1], FP32R, name="onescol_r")
        nc.scalar.activation(st["onescol_r"][:], onescol_f[:], AF.Copy)
        st["ones3_f"] = pers.tile([128, 3], FP32, name="ones3_f")
        nc.vector.memset(st["ones3_f"][:], 1.0)
        st["eps_sb"] = pers.tile([128, 1], FP32, name="eps_sb")
        nc.vector.memset(st["eps_sb"][:], EPS)
        st["gflag_sb"] = pers.tile([128, 1], FP32, name="gflag_sb")
        nc.sync.dma_start(st["gflag_sb"][:], dr["gflag"].ap())

        st["mod_g"] = pers.tile([128, 36], FP32, name="mod_g")
        st["bcast"] = {m: pers.tile([128, D], FP32, name=f"bcast{m}")
                       for m in (2, 3, 4, 5)}
        st["ar_in"] = dram.tile([128, 72], FP32, name="ar_in")
        st["ar_out"] = dram.tile([128, 72], FP32, name="ar_out")
        rs_in = dram.tile([L, D], FP32, name="rs_in")
        rs_out = dram.tile([TOK, D], FP32, name="rs_out")

        _phase_a(nc, tc, dr, pers, st)

        with tc.tile_pool(name="p_qv", bufs=1) as p_qv:
            qkr_A = p_qv.tile([128, L], FP32R, name="qkr_A")
            qkr_B = p_qv.tile([128, L], FP32R, name="qkr_B")
            qkr_C = p_qv.tile([64, L], FP32R, name="qkr_C")
            qkr_D = p_qv.tile([64, L], FP32R, name="qkr_D")
            qkr = (qkr_A, qkr_B, qkr_C, qkr_D)
            v_sb = [p_qv.tile([128, 200], FP32R, name=f"v_sb{t}")
                    for t in range(KC)]
            wo_sb = [p_qv.tile([64, D], FP32R, name=f"wo{h}")
                     for h in range(HC)]
            for h in range(HC):
                nc.sync.dma_start(wo_sb[h][:],
                                  dr["wo"].ap()[64 * h:64 * (h + 1), :])
            with tc.tile_pool(name="p_h", bufs=1) as p_h:
                hT = [p_h.tile([128, L], FP32R, name=f"hT{k}")
                      for k in range(DK)]
                _phase_b(nc, tc, dr, st, hT)
                _phase_c(nc, tc, dr, st, hT, qkr, v_sb)
            _attention(nc, tc, st, qkr, v_sb, wo_sb, rs_in)

        nc.gpsimd.collective_compute(
            "ReduceScatter", ALU.add, replica_groups=GROUPS,
            ins=[rs_in.opt()], outs=[rs_out.opt()])
        with tc.tile_pool(name="p_x1", bufs=1) as p_x1:
            x1 = [p_x1.tile([128, D], FP32, name=f"x1_{t}") for t in range(4)]
            with tc.tile_pool(name="px", bufs=2) as px:
                for t in range(4):
                    rsx = px.tile([128, D], FP32, name="rsx")
                    xs = px.tile([128, D], FP32, name="xs")
                    nc.sync.dma_start(rsx[:],
                                      rs_out[128 * t:128 * (t + 1), :])
                    nc.sync.dma_start(
                        xs[:], dr["x_slice"].ap()[128 * t:128 * (t + 1), :])
                    nc.vector.tensor_add(x1[t][:], rsx[:], xs[:])

            _ffn(nc, tc, dr, st, x1, out_d)


# ---------------------------------------------------------------- entry
def get_program(reps=1):
    key = f"nc{reps}"
    if key not in _CACHE:
        _CACHE[key] = build_program(reps)
    return _CACHE[key]


def make_in_maps(inputs):
    cores = host_prep(inputs)
    names = [s[0] for s in DRAM_SPECS]
    return [{n: cores[i][n] for n in names} for i in range(NC_)]


def kernel(**inputs):
    nc = get_program()
    in_maps = make_in_maps(inputs)
    res = bass_utils.run_bass_kernel_spmd(nc, in_maps, list(range(NC_)))
    out = np.zeros((B, L, D), np.float32)
    for i in range(NC_):
        g, r = i // G, i % G
        out[g, TOK * r:TOK * (r + 1)] = res.results[i]["out"]
    return out

